# revision 28
# baseline (speedup 1.0000x reference)
"""Trainium2 Bass kernel for a 2-layer DGCN (graph conv) on 8 NeuronCores.

Reference computation (fp32):
    h1  = relu(IFadj @ (x @ W1) + b1)         # [N, NHID]
    out = BN(adj @ (h1 @ W2) + b2)            # [N, OUTD], BN in eval mode

Distribution: rows of x / IFadj / adj are sharded across 8 cores
(row-parallel graph partitioning). Per core (rows R_k), v6 schedule:

  phase A: S_own = x[R_k] @ W1 (cb-outer over 8 PSUM banks, x/W1
           streamed in interleaved 128-row slabs); two pipelined
           S-AllGather chunks fired as soon as their bounce lands,
           with the triggers pinned to scheduler priority 0. The
           first collective absorbs the cross-core launch-skew
           barrier (~45us, unavoidable) while local compute runs.
  phase B: redundantly compute S for global node blocks 5..7 (same on
           every core, from a replicated x slice): local work that
           covers the barrier + gather window.
  phase C: h1T = relu(S^T @ B + bias), two i-half passes; within each
           pass, local m-blocks 5..7 first (no collective dep), then
           gathered blocks 0..4 chunk-major. After each pass: z-half
           = h1 @ (W2/4) in fp8, Z-AllGather chunk fired mid-kernel.
  phase D: outT = Z-as-lhsT vs adjT_k rhs, fp8e4 DoubleRow matmuls
           (one instruction per adjacent m-tile pair); BN fused on
           the PSUM evict with the x4 range-fold undone in the BN
           scale.

Engine/queue discipline (the v2-v5 lessons): every dma_start
dispatches serially on its issuing engine's queue, and the tile
scheduler may reorder same-engine dispatches, so a collective-gated
DMA can head-block urgent loads behind it. Therefore:
  - SP (nc.sync) queue: compute-paced setup only (w1/x/xTr slabs,
    bounce writes, consts, out writes).
  - Activation (nc.scalar) queue: the big streaming loads (IFadj
    dual-m-tile tiles, adj pair tiles) -- never gated on collectives.
  - GpSimd software DGE: ONLY the collective-gated staging (gathered
    S quarters, gathered z blocks); a head-block there delays nothing
    else. The S-AllGather triggers (also gpsimd) carry priority 0 so
    staging can never be scheduled ahead of them.
  - Plain-Copy PSUM evictions run on the Vector engine, keeping the
    Activation engine free for relu evictions + its DMA queue.
All transfers are plain 2D/3D slices: the host pre-permutes IFadj and
adj into pair-interleaved layouts, W2 partition-major, and the z
bounce is written p-major so gathered z is plain-sliceable.

Precision: layer-1 runs lhsT=S in bf16 against the IFadj moving
operand in fp8e4, CENTERED: B = IFadj - 1/2 (entries U[0,1] ->
[-1/2,1/2]); the exact mean term 1/2*colsum(S)_j is folded into the
layer-1 bias on the host (colsum(S) = (sum_m x[m,:]) @ W1, a trivial
host matvec). Measured end-to-end rel err ~2.2e-3 vs the 2e-2 gate.
The layer-2 spmm runs both operands fp8e4 with z pre-scaled by 1/4 to
sit far inside e4m3 range.
"""

import numpy as np
import ml_dtypes

NCORES = 8
N = 8192
NFEAT = 1024
NHID = 512
OUTD = 256
ROWS = N // NCORES  # 1024
P = 128
BN_EPS = 1e-5

CB = NFEAT // P   # 8  c-blocks (x feature contraction)
IB = ROWS // P    # 8  i-blocks per node block
JB = NHID // P    # 4  j-blocks (hidden)
MT = N // P       # 64 m-tiles (global node contraction)
HF = 512          # matmul moving free dim (PSUM bank limit)
IH = ROWS // HF   # 2 i-halves of the local row range
OB = OUTD // P    # 2 output-feature blocks
GC = 2            # allgather chunks (S and Z)
QT = 4            # m-tiles per (chunk, block) quarter
NRED = 3          # redundant S blocks (global blocks 5..7)
RED0 = NCORES - NRED  # first redundant block = 5

_BF16 = ml_dtypes.bfloat16
_F8 = ml_dtypes.float8_e4m3

_cache = {}


def _build():
    import concourse.mybir as mybir
    import concourse.tile as tile
    from concourse import bacc

    dt = mybir.dt
    f32 = dt.float32
    bf16 = dt.bfloat16
    f8 = dt.float8e4
    AF = mybir.ActivationFunctionType
    DR = mybir.MatmulPerfMode.DoubleRow
    MULT = mybir.AluOpType.mult

    nc = bacc.Bacc("TRN2", target_bir_lowering=False, debug=False,
                   num_devices=NCORES)

    # x^T partition-major: [P, CB*ROWS], one plain DMA loads it all
    xT_e = nc.dram_tensor("xT", [P, CB * ROWS], bf16, kind="ExternalInput")
    # replicated x rows for global node blocks 5..7, partition-major
    xTr_e = nc.dram_tensor("xTr", [P, CB * NRED * ROWS], bf16,
                           kind="ExternalInput")
    # centered IFadj^T in fp8, pair-interleaved and split by column
    # half on the host: row ih*4096 + pair*P + p, col t*HF + c holds
    # IFadjT[(2*pair+t)*P + p, ih*HF + c] - 1/2
    ifadjH_e = nc.dram_tensor("ifadjH", [IH * N // 2, 2 * HF], f8,
                              kind="ExternalInput")
    # adj rows pair-interleaved on host: row pair*P+p holds m-tiles
    # (2*pair, 2*pair+1) side by side -> [P, 2, ROWS] is a plain slice
    adjP_e = nc.dram_tensor("adjP", [N // 2, 2 * ROWS], f8,
                            kind="ExternalInput")
    # W1 partition-major: [P, CB*NHID]
    w1_e = nc.dram_tensor("w1", [P, CB * NHID], bf16, kind="ExternalInput")
    # W2/4, partition-major: [P, JB*OUTD]
    w2_e = nc.dram_tensor("w2", [P, JB * OUTD], bf16, kind="ExternalInput")
    # layer-1 bias + 1/2*colsum(S) fold, [P, JB]
    b1p_e = nc.dram_tensor("b1p", [P, JB], f32, kind="ExternalInput")
    bnsc_e = nc.dram_tensor("bnsc", [P, OB], f32, kind="ExternalInput")
    bnbi_e = nc.dram_tensor("bnbi", [P, OB], f32, kind="ExternalInput")
    # outT: [OUTD, ROWS]; the host transposes each core's block.
    out_e = nc.dram_tensor("out", [OUTD, ROWS], f32, kind="ExternalOutput")

    groups = [list(range(NCORES))]

    def allgather(g_in, g_out):
        nc.gpsimd.collective_compute(
            "AllGather", mybir.AluOpType.bypass, replica_groups=groups,
            ins=[g_in[:]], outs=[g_out[:]])

    with tile.TileContext(nc) as tc:
        with (
            tc.tile_pool(name="const", bufs=1) as const,
            tc.tile_pool(name="xres", bufs=1) as xres_p,
            tc.tile_pool(name="sloc", bufs=1) as sloc_p,
            tc.tile_pool(name="sred", bufs=1) as sred_p,
            tc.tile_pool(name="sgt", bufs=12) as sgt_p,
            tc.tile_pool(name="h1", bufs=1) as h1_p,
            tc.tile_pool(name="zsb", bufs=1) as z_p,
            tc.tile_pool(name="zchunk", bufs=8) as zchunk_p,
            tc.tile_pool(name="astream", bufs=8) as astream,
            tc.tile_pool(name="apair", bufs=6) as apair_p,
            tc.tile_pool(name="outsb", bufs=1) as outsb_p,
            tc.tile_pool(name="dram", bufs=1, space="DRAM") as dram,
        ):
            # ---- DRAM bounce buffers for the collectives
            RPC = ROWS // GC  # rows bounced per S chunk (512)
            s_bounce = [dram.tile([RPC, NHID], bf16, name=f"sb{c}")
                        for c in range(GC)]
            # s_all[c] row k*RPC + r = S[global row k*ROWS + c*RPC + r]
            s_all = [dram.tile([RPC * NCORES, NHID], bf16,
                               addr_space="Shared", name=f"sa{c}")
                     for c in range(GC)]
            # z bounce is p-major: row p holds (t, o), t = chunk-local
            # i-block -> gathered z is plain-sliceable per core block
            z_bounce = [dram.tile([P, QT * OUTD], f8, name=f"zb{c}")
                        for c in range(GC)]
            z_all = [dram.tile([P * NCORES, QT * OUTD], f8,
                               addr_space="Shared", name=f"za{c}")
                     for c in range(GC)]

            s_loc = sloc_p.tile([P, IB, NHID], bf16)
            s_red = sred_p.tile([P, NRED * IB, NHID], bf16)
            w1_sb = const.tile([P, CB, NHID], bf16)

            # ---- phase A: own S block in two i-half sub-passes so the
            # first AllGather chunk fires at ~half-A. x/W1/xTr are
            # host-permuted partition-major, so each loads with one
            # big plain DMA (SP dispatch slots are a serial resource).
            xT_sb = xres_p.tile([P, CB, ROWS], bf16)
            xTr_sb = xres_p.tile([P, CB, NRED * ROWS], bf16)
            with tc.tile_pool(name="psA", bufs=1, space="PSUM") as psA:
                nc.sync.dma_start(w1_sb[:], w1_e[:])
                nc.sync.dma_start(xT_sb[:], xT_e[:])
                nc.sync.dma_start(xTr_sb[:], xTr_e[:])
                for c in range(GC):
                    ps_own = [psA.tile([P, NHID], f32, name=f"pso{c}_{t}",
                                       tag=f"pa{t}")
                              for t in range(IB // GC)]
                    for cb in range(CB):
                        for t in range(IB // GC):
                            ib = c * (IB // GC) + t
                            nc.tensor.matmul(
                                ps_own[t][:],
                                xT_sb[:, cb, ib * P:(ib + 1) * P],
                                w1_sb[:, cb, :],
                                start=(cb == 0), stop=(cb == CB - 1))
                    for t in range(IB // GC):
                        ib = c * (IB // GC) + t
                        nc.vector.tensor_scalar(
                            s_loc[:, ib, :], ps_own[t][:], 1.0, None,
                            MULT)
                        nc.sync.dma_start(
                            s_bounce[c][t * P:(t + 1) * P, :],
                            s_loc[:, ib, :])
                    with tc.high_priority():
                        allgather(s_bounce[c], s_all[c])

                # remaining constants (needed from phase C on)
                b1p_sb = const.tile([P, JB], f32)
                nc.sync.dma_start(b1p_sb[:], b1p_e[:])
                w2_sb = const.tile([P, JB, OUTD], bf16)
                nc.sync.dma_start(w2_sb[:], w2_e[:])
                bnsc_sb = const.tile([P, OB], f32)
                nc.sync.dma_start(bnsc_sb[:], bnsc_e[:])
                bnbi_sb = const.tile([P, OB], f32)
                nc.sync.dma_start(bnbi_sb[:], bnbi_e[:])

                # ---- phase B: redundant S for global blocks 5..7
                for r in range(NRED):
                    ps_r = [psA.tile([P, NHID], f32, name=f"psr{r}_{ib}",
                                     tag=f"pa{ib}")
                            for ib in range(IB)]
                    for cb in range(CB):
                        for ib in range(IB):
                            nc.tensor.matmul(
                                ps_r[ib][:],
                                xTr_sb[:, cb,
                                       (r * IB + ib) * P:
                                       (r * IB + ib + 1) * P],
                                w1_sb[:, cb, :],
                                start=(cb == 0), stop=(cb == CB - 1))
                    for ib in range(IB):
                        nc.vector.tensor_scalar(
                            s_red[:, r * IB + ib, :], ps_r[ib][:], 1.0,
                            None, MULT)

            h1T = h1_p.tile([P, JB, ROWS], bf16)
            z_sb = z_p.tile([P, IB, OUTD], f8)

            # m-traversal per pass, in adjacent PAIRS (one IFadj dual
            # tile feeds two m-tiles): local blocks 5..7 first, then
            # gathered blocks chunk-major.
            pair_walk = ([(g, q) for g in range(RED0, NCORES)
                          for q in range(0, IB, 2)]
                         + [(g, c * QT + t) for c in range(GC)
                            for g in range(RED0)
                            for t in range(0, QT, 2)])

            # ---- phase C, i-half pass ih. Streaming dual tiles on the
            # Activation queue; gathered-S staging on the gpsimd DGE.
            def l1_pass(ih, psh, psz):
                psum_h = [psh.tile([P, HF], f32, name=f"ph{jb}_{ih}",
                                   tag=f"ph{jb}")
                          for jb in range(JB)]
                s_gt = {}
                n_emitted = 0
                for g, q in pair_walk:
                    a_dual = astream.tile([P, 2, HF], f8, tag="adual")
                    pidx = (g * IB + q) // 2
                    nc.scalar.dma_start(
                        a_dual[:],
                        ifadjH_e[ih * (N // 2) + pidx * P:
                                 ih * (N // 2) + (pidx + 1) * P, :])
                    for u in range(2):
                        qq = q + u
                        if g >= RED0:
                            s_src = s_red[:, (g - RED0) * IB + qq, :]
                        else:
                            if s_gt.get((g, qq)) is None:
                                c, t = divmod(qq, QT)
                                st = sgt_p.tile([P, NHID], bf16,
                                                name=f"sg{ih}_{g}_{qq}",
                                                tag="sgt")
                                nc.gpsimd.dma_start(
                                    st[:],
                                    s_all[c][(g * QT + t) * P:
                                             (g * QT + t + 1) * P, :])
                                s_gt[(g, qq)] = st
                            s_src = s_gt[(g, qq)][:]
                        for jb in range(JB):
                            nc.tensor.matmul(
                                psum_h[jb][:],
                                s_src[:, jb * P:(jb + 1) * P],
                                a_dual[:, u, :],
                                start=(n_emitted == 0),
                                stop=(n_emitted == MT - 1),
                            )
                        n_emitted += 1
                # epilogue: relu + (b1 + colsum/2) bias into h1T half
                for jb in range(JB):
                    nc.scalar.activation(
                        h1T[:, jb, ih * HF:(ih + 1) * HF],
                        psum_h[jb][:], AF.Relu,
                        bias=b1p_sb[:, jb:jb + 1])
                # z for this half's i-blocks (fp8, W2 pre-scaled by 1/4),
                # p-major bounce, gather chunk ih
                for t in range(IB // IH):
                    ib = ih * (IB // IH) + t
                    ps = psz.tile([P, OUTD], f32, tag="z")
                    for jb in range(JB):
                        nc.tensor.matmul(
                            ps[:],
                            h1T[:, jb, ib * P:(ib + 1) * P],
                            w2_sb[:, jb, :],
                            start=(jb == 0), stop=(jb == JB - 1),
                        )
                    nc.vector.tensor_scalar(
                        z_sb[:, ib, :], ps[:], 1.0, None, MULT)
                    nc.sync.dma_start(
                        z_bounce[ih][:, t * OUTD:(t + 1) * OUTD],
                        z_sb[:, ib, :])
                allgather(z_bounce[ih], z_all[ih])

            with (
                tc.tile_pool(name="psh", bufs=1, space="PSUM") as psh,
                tc.tile_pool(name="psz", bufs=2, space="PSUM") as psz,
            ):
                for ih in range(IH):
                    l1_pass(ih, psh, psz)

            # ---- phase D: outT[o, i] = sum_m Z[m, o] * adjT[m, i]
            # fp8 DoubleRow, one matmul per adjacent m-tile pair.
            # z_all[c] row k*P+p holds (t, o) = z[k*ROWS + c*RPC + t*P+p]
            # -> m-tile of (c, k, t) is 8k + 4c + t.
            outT_sb = outsb_p.tile([P, OB, ROWS], f32)
            with tc.tile_pool(name="ps4", bufs=1, space="PSUM") as ps4:
                psum_o = [[ps4.tile([P, HF], f32, name=f"po{ob}_{ih}",
                                    tag=f"po{ob}_{ih}")
                           for ih in range(IH)] for ob in range(OB)]
                first = True
                for c in range(GC):
                    for k in range(NCORES):
                        zc_sb = zchunk_p.tile([P, QT, OUTD], f8,
                                              tag="zchunk")
                        nc.gpsimd.dma_start(
                            zc_sb[:], z_all[c][k * P:(k + 1) * P, :])
                        last_grp = (c == GC - 1 and k == NCORES - 1)
                        a_prs = {}
                        for pr in range(0, QT, 2):
                            mt = IB * k + QT * c + pr
                            a_pair = apair_p.tile([P, 2, ROWS], f8,
                                                  tag="apair")
                            nc.scalar.dma_start(
                                a_pair[:],
                                adjP_e[(mt // 2) * P:(mt // 2 + 1) * P, :])
                            a_prs[pr] = a_pair
                        # last group runs ob-outer so ob=0's accumulators
                        # stop early and BN eviction overlaps ob=1 matmuls
                        if last_grp:
                            for ob in range(OB):
                                for pr in range(0, QT, 2):
                                    for ih in range(IH):
                                        nc.tensor.matmul(
                                            psum_o[ob][ih][:],
                                            zc_sb[:, pr:pr + 2,
                                                  ob * P:(ob + 1) * P],
                                            a_prs[pr][:, :,
                                                      ih * HF:
                                                      (ih + 1) * HF],
                                            start=False,
                                            stop=(pr == QT - 2),
                                            perf_mode=DR,
                                        )
                        else:
                            for pr in range(0, QT, 2):
                                for ob in range(OB):
                                    for ih in range(IH):
                                        nc.tensor.matmul(
                                            psum_o[ob][ih][:],
                                            zc_sb[:, pr:pr + 2,
                                                  ob * P:(ob + 1) * P],
                                            a_prs[pr][:, :,
                                                      ih * HF:
                                                      (ih + 1) * HF],
                                            start=first, stop=False,
                                            perf_mode=DR,
                                        )
                                first = False
                # fused BN affine on PSUM evict: out = psum*scale + bias
                for ob in range(OB):
                    for ih in range(IH):
                        nc.vector.tensor_scalar(
                            outT_sb[:, ob, ih * HF:(ih + 1) * HF],
                            psum_o[ob][ih][:],
                            bnsc_sb[:, ob:ob + 1],
                            bnbi_sb[:, ob:ob + 1],
                            mybir.AluOpType.mult,
                            mybir.AluOpType.add)
                    nc.sync.dma_start(
                        out_e[ob * P:(ob + 1) * P, :], outT_sb[:, ob, :])

    nc.compile()
    return nc


def _get_nc():
    if "nc" not in _cache:
        _cache["nc"] = _build()
    return _cache["nc"]


def kernel(x, IFadj, adj, W1, b1, W2, b2, bn_gamma, bn_beta, bn_mean, bn_var):
    from concourse.bass_utils import run_bass_kernel_spmd

    x = np.asarray(x, dtype=np.float32)
    IFadj = np.asarray(IFadj, dtype=np.float32)
    adj = np.asarray(adj, dtype=np.float32)
    W1 = np.asarray(W1, dtype=np.float32)
    b1 = np.asarray(b1, dtype=np.float32)
    W2 = np.asarray(W2, dtype=np.float32)
    b2 = np.asarray(b2, dtype=np.float32)
    bn_gamma = np.asarray(bn_gamma, dtype=np.float32)
    bn_beta = np.asarray(bn_beta, dtype=np.float32)
    bn_mean = np.asarray(bn_mean, dtype=np.float32)
    bn_var = np.asarray(bn_var, dtype=np.float32)

    # host-side prep: shard rows, transpose for PE lhsT layout, cast.
    # W2 is pre-scaled by 1/4 so z stays well inside fp8e4 range; the
    # BN scale is multiplied by 4 to undo it after the layer-2 spmm.
    w1b = np.ascontiguousarray(
        W1.astype(_BF16).reshape(CB, P, NHID).transpose(1, 0, 2)
        .reshape(P, CB * NHID))
    w2b = np.ascontiguousarray(
        (W2 * 0.25).astype(_BF16).reshape(JB, P, OUTD)
        .transpose(1, 0, 2).reshape(P, JB * OUTD))
    # layer-1 bias including the exact 1/2*colsum(S) centering term
    colsum = x.sum(axis=0, dtype=np.float64).astype(np.float32) @ W1
    b1c = b1 + 0.5 * colsum
    b1p = np.ascontiguousarray(b1c.reshape(JB, P).T)  # [P, JB]
    inv = bn_gamma / np.sqrt(bn_var + BN_EPS)
    bias_tot = b2 * inv + bn_beta - bn_mean * inv
    bnsc = np.ascontiguousarray((4.0 * inv).reshape(OB, P).T)   # [P, OB]
    bnbi = np.ascontiguousarray(bias_tot.reshape(OB, P).T)      # [P, OB]

    # replicated x rows for global node blocks 5..7, partition-major
    xTr = np.ascontiguousarray(
        x[RED0 * ROWS:].T.astype(_BF16).reshape(CB, P, NRED * ROWS)
        .transpose(1, 0, 2).reshape(P, CB * NRED * ROWS))

    in_maps = []
    for k in range(NCORES):
        r0, r1 = k * ROWS, (k + 1) * ROWS
        # centered IFadj^T in fp8: [m, col] -> [ih, pair, p, t, c]
        A8 = (IFadj[r0:r1].T - np.float32(0.5)).astype(_F8)  # [N, ROWS]
        ifadjH = np.ascontiguousarray(
            A8.reshape(N // 256, 2, P, IH, HF).transpose(3, 0, 2, 1, 4)
            .reshape(IH * N // 2, 2 * HF))
        adjT8 = np.ascontiguousarray(adj[r0:r1].T).astype(_F8)  # [N, ROWS]
        # pair-interleave: row pair*P+p = m-tiles (2p, 2p+1) side by side
        adjP = np.ascontiguousarray(
            adjT8.reshape(N // 256, 2, P, ROWS).transpose(0, 2, 1, 3)
            .reshape(N // 2, 2 * ROWS))
        xTp = np.ascontiguousarray(
            x[r0:r1].T.astype(_BF16).reshape(CB, P, ROWS)
            .transpose(1, 0, 2).reshape(P, CB * ROWS))
        in_maps.append({
            "xT": xTp,
            "xTr": xTr,
            "ifadjH": ifadjH,
            "adjP": adjP,
            "w1": w1b,
            "w2": w2b,
            "b1p": b1p,
            "bnsc": bnsc,
            "bnbi": bnbi,
        })

    global _last_in_maps
    _last_in_maps = in_maps

    nc = _get_nc()
    try:
        res = run_bass_kernel_spmd(nc, in_maps, list(range(NCORES)))
    except Exception:
        # transient device wedge (NRT_EXEC_UNIT_UNRECOVERABLE etc.) --
        # a straight retry has been observed to recover
        import time
        time.sleep(2.0)
        res = run_bass_kernel_spmd(nc, in_maps, list(range(NCORES)))
    # per-core output is outT [OUTD, ROWS]; transpose back and stack rows
    return np.concatenate(
        [np.ascontiguousarray(res.results[k]["out"].T)
         for k in range(NCORES)], axis=0)


# revision 34
# speedup vs baseline: 1.0200x; 1.0200x over previous
"""Trainium2 Bass kernel for a 2-layer DGCN (graph conv) on 8 NeuronCores.

Reference computation (fp32):
    h1  = relu(IFadj @ (x @ W1) + b1)         # [N, NHID]
    out = BN(adj @ (h1 @ W2) + b2)            # [N, OUTD], BN in eval mode

Distribution: rows of x / IFadj / adj are sharded across 8 cores
(row-parallel graph partitioning). Per core (rows R_k), v6 schedule:

  phase A: S_own = x[R_k] @ W1 (cb-outer over 8 PSUM banks, x/W1
           streamed in interleaved 128-row slabs); two pipelined
           S-AllGather chunks fired as soon as their bounce lands,
           with the triggers pinned to scheduler priority 0. The
           first collective absorbs the cross-core launch-skew
           barrier (~45us, unavoidable) while local compute runs.
  phase B: redundantly compute S for global node blocks 5..7 (same on
           every core, from a replicated x slice): local work that
           covers the barrier + gather window.
  phase C: h1T = relu(S^T @ B + bias), two i-half passes; within each
           pass, local m-blocks 5..7 first (no collective dep), then
           gathered blocks 0..4 chunk-major. After each pass: z-half
           = h1 @ (W2/4) in fp8, Z-AllGather chunk fired mid-kernel.
  phase D: outT = Z-as-lhsT vs adjT_k rhs, fp8e4 DoubleRow matmuls
           (one instruction per adjacent m-tile pair); BN fused on
           the PSUM evict with the x4 range-fold undone in the BN
           scale.

Engine/queue discipline (the v2-v5 lessons): every dma_start
dispatches serially on its issuing engine's queue, and the tile
scheduler may reorder same-engine dispatches, so a collective-gated
DMA can head-block urgent loads behind it. Therefore:
  - SP (nc.sync) queue: compute-paced setup only (w1/x/xTr slabs,
    bounce writes, consts, out writes).
  - Activation (nc.scalar) queue: the big streaming loads (IFadj
    dual-m-tile tiles, adj pair tiles) -- never gated on collectives.
  - GpSimd software DGE: ONLY the collective-gated staging (gathered
    S quarters, gathered z blocks); a head-block there delays nothing
    else. The S-AllGather triggers (also gpsimd) carry priority 0 so
    staging can never be scheduled ahead of them.
  - Plain-Copy PSUM evictions run on the Vector engine, keeping the
    Activation engine free for relu evictions + its DMA queue.
All transfers are plain 2D/3D slices: the host pre-permutes IFadj and
adj into pair-interleaved layouts, W2 partition-major, and the z
bounce is written p-major so gathered z is plain-sliceable.

Precision: layer-1 runs lhsT=S in bf16 against the IFadj moving
operand in fp8e4, CENTERED: B = IFadj - 1/2 (entries U[0,1] ->
[-1/2,1/2]); the exact mean term 1/2*colsum(S)_j is folded into the
layer-1 bias on the host (colsum(S) = (sum_m x[m,:]) @ W1, a trivial
host matvec). Measured end-to-end rel err ~2.2e-3 vs the 2e-2 gate.
The layer-2 spmm runs both operands fp8e4 with z pre-scaled by 1/4 to
sit far inside e4m3 range.
"""

import numpy as np
import ml_dtypes

NCORES = 8
N = 8192
NFEAT = 1024
NHID = 512
OUTD = 256
ROWS = N // NCORES  # 1024
P = 128
BN_EPS = 1e-5

CB = NFEAT // P   # 8  c-blocks (x feature contraction)
IB = ROWS // P    # 8  i-blocks per node block
JB = NHID // P    # 4  j-blocks (hidden)
MT = N // P       # 64 m-tiles (global node contraction)
HF = 512          # matmul moving free dim (PSUM bank limit)
IH = ROWS // HF   # 2 i-halves of the local row range
OB = OUTD // P    # 2 output-feature blocks
GC = 2            # allgather chunks (S and Z)
QT = 4            # m-tiles per (chunk, block) quarter
NRED = 3          # redundant S blocks (global blocks 5..7)
RED0 = NCORES - NRED  # first redundant block = 5

_BF16 = ml_dtypes.bfloat16
_F8 = ml_dtypes.float8_e4m3

_cache = {}


def _build():
    import concourse.mybir as mybir
    import concourse.tile as tile
    from concourse import bacc

    dt = mybir.dt
    f32 = dt.float32
    bf16 = dt.bfloat16
    f8 = dt.float8e4
    AF = mybir.ActivationFunctionType
    DR = mybir.MatmulPerfMode.DoubleRow
    MULT = mybir.AluOpType.mult

    nc = bacc.Bacc("TRN2", target_bir_lowering=False, debug=False,
                   num_devices=NCORES)

    # x^T partition-major: [P, CB*ROWS], one plain DMA loads it all
    xT_e = nc.dram_tensor("xT", [P, CB * ROWS], bf16, kind="ExternalInput")
    # replicated x rows for global node blocks 5..7, partition-major
    xTr_e = nc.dram_tensor("xTr", [P, CB * NRED * ROWS], bf16,
                           kind="ExternalInput")
    # centered IFadj^T in fp8, pair-interleaved and split by column
    # half on the host: row ih*4096 + pair*P + p, col t*HF + c holds
    # IFadjT[(2*pair+t)*P + p, ih*HF + c] - 1/2
    ifadjH_e = nc.dram_tensor("ifadjH", [IH * N // 2, 2 * HF], f8,
                              kind="ExternalInput")
    # adj rows pair-interleaved on host: row pair*P+p holds m-tiles
    # (2*pair, 2*pair+1) side by side -> [P, 2, ROWS] is a plain slice
    adjP_e = nc.dram_tensor("adjP", [N // 2, 2 * ROWS], f8,
                            kind="ExternalInput")
    # W1 partition-major: [P, CB*NHID]
    w1_e = nc.dram_tensor("w1", [P, CB * NHID], bf16, kind="ExternalInput")
    # W2/4, partition-major: [P, JB*OUTD]
    w2_e = nc.dram_tensor("w2", [P, JB * OUTD], bf16, kind="ExternalInput")
    # layer-1 bias + 1/2*colsum(S) fold, [P, JB]
    b1p_e = nc.dram_tensor("b1p", [P, JB], f32, kind="ExternalInput")
    bnsc_e = nc.dram_tensor("bnsc", [P, OB], f32, kind="ExternalInput")
    bnbi_e = nc.dram_tensor("bnbi", [P, OB], f32, kind="ExternalInput")
    # outT: [OUTD, ROWS]; the host transposes each core's block.
    out_e = nc.dram_tensor("out", [OUTD, ROWS], f32, kind="ExternalOutput")

    groups = [list(range(NCORES))]

    def allgather(g_in, g_out):
        nc.gpsimd.collective_compute(
            "AllGather", mybir.AluOpType.bypass, replica_groups=groups,
            ins=[g_in[:]], outs=[g_out[:]])

    with tile.TileContext(nc) as tc:
        with (
            tc.tile_pool(name="const", bufs=1) as const,
            tc.tile_pool(name="xres", bufs=1) as xres_p,
            tc.tile_pool(name="sloc", bufs=1) as sloc_p,
            tc.tile_pool(name="sred", bufs=1) as sred_p,
            tc.tile_pool(name="sgt", bufs=12) as sgt_p,
            tc.tile_pool(name="h1", bufs=1) as h1_p,
            tc.tile_pool(name="zsb", bufs=1) as z_p,
            tc.tile_pool(name="zchunk", bufs=8) as zchunk_p,
            tc.tile_pool(name="astream", bufs=8) as astream,
            tc.tile_pool(name="apair", bufs=6) as apair_p,
            tc.tile_pool(name="outsb", bufs=1) as outsb_p,
            tc.tile_pool(name="dram", bufs=1, space="DRAM") as dram,
        ):
            # ---- DRAM bounce buffers for the collectives
            RPC = ROWS // GC  # rows bounced per S chunk (512)
            s_bounce = [dram.tile([RPC, NHID], bf16, name=f"sb{c}")
                        for c in range(GC)]
            # s_all[c] row k*RPC + r = S[global row k*ROWS + c*RPC + r]
            s_all = [dram.tile([RPC * NCORES, NHID], bf16,
                               addr_space="Shared", name=f"sa{c}")
                     for c in range(GC)]
            # z bounce is p-major: row p holds (t, o), t = chunk-local
            # i-block -> gathered z is plain-sliceable per core block
            z_bounce = [dram.tile([P, QT * OUTD], f8, name=f"zb{c}")
                        for c in range(GC)]
            z_all = [dram.tile([P * NCORES, QT * OUTD], f8,
                               addr_space="Shared", name=f"za{c}")
                     for c in range(GC)]

            s_loc = sloc_p.tile([P, IB, NHID], bf16)
            s_red = sred_p.tile([P, NRED * IB, NHID], bf16)
            w1_sb = const.tile([P, CB, NHID], bf16)

            # ---- phase A: own S block in two i-half sub-passes so the
            # first AllGather chunk fires at ~half-A. x/W1/xTr are
            # host-permuted partition-major, so each loads with one
            # big plain DMA (SP dispatch slots are a serial resource).
            xT_sb = xres_p.tile([P, CB, ROWS], bf16)
            xTr_sb = xres_p.tile([P, CB, NRED * ROWS], bf16)
            with tc.tile_pool(name="psA", bufs=1, space="PSUM") as psA:
                hw = CB // 2
                for h in range(2):
                    nc.sync.dma_start(
                        w1_sb[:, h * hw:(h + 1) * hw, :],
                        w1_e[:, h * hw * NHID:(h + 1) * hw * NHID])
                    nc.sync.dma_start(
                        xT_sb[:, h * hw:(h + 1) * hw, :],
                        xT_e[:, h * hw * ROWS:(h + 1) * hw * ROWS])
                for h in range(2):
                    nc.sync.dma_start(
                        xTr_sb[:, h * hw:(h + 1) * hw, :],
                        xTr_e[:, h * hw * NRED * ROWS:
                              (h + 1) * hw * NRED * ROWS])
                for c in range(GC):
                    ps_own = [psA.tile([P, NHID], f32, name=f"pso{c}_{t}",
                                       tag=f"pa{t}")
                              for t in range(IB // GC)]
                    for cb in range(CB):
                        for t in range(IB // GC):
                            ib = c * (IB // GC) + t
                            nc.tensor.matmul(
                                ps_own[t][:],
                                xT_sb[:, cb, ib * P:(ib + 1) * P],
                                w1_sb[:, cb, :],
                                start=(cb == 0), stop=(cb == CB - 1))
                    for t in range(IB // GC):
                        ib = c * (IB // GC) + t
                        nc.vector.tensor_scalar(
                            s_loc[:, ib, :], ps_own[t][:], 1.0, None,
                            MULT)
                        nc.sync.dma_start(
                            s_bounce[c][t * P:(t + 1) * P, :],
                            s_loc[:, ib, :])
                    with tc.high_priority():
                        allgather(s_bounce[c], s_all[c])

                # remaining constants (needed from phase C on)
                b1p_sb = const.tile([P, JB], f32)
                nc.sync.dma_start(b1p_sb[:], b1p_e[:])
                w2_sb = const.tile([P, JB, OUTD], bf16)
                nc.sync.dma_start(w2_sb[:], w2_e[:])
                bnsc_sb = const.tile([P, OB], f32)
                nc.sync.dma_start(bnsc_sb[:], bnsc_e[:])
                bnbi_sb = const.tile([P, OB], f32)
                nc.sync.dma_start(bnbi_sb[:], bnbi_e[:])

                # ---- phase B: redundant S for global blocks 5..7, in
                # 4-bank sub-passes so evictions overlap the next
                # sub-pass's matmuls
                for r in range(NRED):
                    for hh in range(2):
                        ps_r = [psA.tile([P, NHID], f32,
                                         name=f"psr{r}_{hh}_{t}",
                                         tag=f"pa{hh * 4 + t}")
                                for t in range(4)]
                        for cb in range(CB):
                            for t in range(4):
                                ib = r * IB + hh * 4 + t
                                nc.tensor.matmul(
                                    ps_r[t][:],
                                    xTr_sb[:, cb, ib * P:(ib + 1) * P],
                                    w1_sb[:, cb, :],
                                    start=(cb == 0), stop=(cb == CB - 1))
                        for t in range(4):
                            nc.vector.tensor_scalar(
                                s_red[:, r * IB + hh * 4 + t, :],
                                ps_r[t][:], 1.0, None, MULT)

            h1T = h1_p.tile([P, JB, ROWS], bf16)
            z_sb = z_p.tile([P, IB, OUTD], f8)

            # m-traversal per pass, in adjacent PAIRS (one IFadj dual
            # tile feeds two m-tiles): local blocks 5..7 first, then
            # gathered blocks chunk-major.
            pair_walk = ([(g, q) for g in range(RED0, NCORES)
                          for q in range(0, IB, 2)]
                         + [(g, c * QT + t) for c in range(GC)
                            for g in range(RED0)
                            for t in range(0, QT, 2)])

            # ---- phase C, i-half pass ih. Streaming dual tiles on the
            # Activation queue; gathered-S staging on the gpsimd DGE.
            def l1_pass(ih, psh, psz):
                psum_h = [psh.tile([P, HF], f32, name=f"ph{jb}_{ih}",
                                   tag=f"ph{jb}")
                          for jb in range(JB)]
                s_gt = {}
                n_emitted = 0
                for g, q in pair_walk:
                    a_dual = astream.tile([P, 2, HF], f8, tag="adual")
                    pidx = (g * IB + q) // 2
                    nc.scalar.dma_start(
                        a_dual[:],
                        ifadjH_e[ih * (N // 2) + pidx * P:
                                 ih * (N // 2) + (pidx + 1) * P, :])
                    for u in range(2):
                        qq = q + u
                        if g >= RED0:
                            s_src = s_red[:, (g - RED0) * IB + qq, :]
                        else:
                            if s_gt.get((g, qq)) is None:
                                c, t = divmod(qq, QT)
                                st = sgt_p.tile([P, NHID], bf16,
                                                name=f"sg{ih}_{g}_{qq}",
                                                tag="sgt")
                                nc.gpsimd.dma_start(
                                    st[:],
                                    s_all[c][(g * QT + t) * P:
                                             (g * QT + t + 1) * P, :])
                                s_gt[(g, qq)] = st
                            s_src = s_gt[(g, qq)][:]
                        for jb in range(JB):
                            nc.tensor.matmul(
                                psum_h[jb][:],
                                s_src[:, jb * P:(jb + 1) * P],
                                a_dual[:, u, :],
                                start=(n_emitted == 0),
                                stop=(n_emitted == MT - 1),
                            )
                        n_emitted += 1
                # epilogue: relu(psum + bias) evicted on the Vector
                # engine per jb, with the z partial matmuls for that jb
                # pipelined right behind (4 i-block accumulators)
                zps = [psz.tile([P, OUTD], f32, name=f"zp{ih}_{t}",
                                tag=f"z{t}")
                       for t in range(IB // IH)]
                for jb in range(JB):
                    nc.vector.tensor_scalar(
                        h1T[:, jb, ih * HF:(ih + 1) * HF],
                        psum_h[jb][:], b1p_sb[:, jb:jb + 1], 0.0,
                        mybir.AluOpType.add, mybir.AluOpType.max)
                    for t in range(IB // IH):
                        ib = ih * (IB // IH) + t
                        nc.tensor.matmul(
                            zps[t][:],
                            h1T[:, jb, ib * P:(ib + 1) * P],
                            w2_sb[:, jb, :],
                            start=(jb == 0), stop=(jb == JB - 1),
                        )
                for t in range(IB // IH):
                    ib = ih * (IB // IH) + t
                    nc.vector.tensor_scalar(
                        z_sb[:, ib, :], zps[t][:], 1.0, None, MULT)
                    nc.sync.dma_start(
                        z_bounce[ih][:, t * OUTD:(t + 1) * OUTD],
                        z_sb[:, ib, :])
                allgather(z_bounce[ih], z_all[ih])

            with (
                tc.tile_pool(name="psh", bufs=1, space="PSUM") as psh,
                tc.tile_pool(name="psz", bufs=1, space="PSUM") as psz,
            ):
                for ih in range(IH):
                    l1_pass(ih, psh, psz)

            # ---- phase D: outT[o, i] = sum_m Z[m, o] * adjT[m, i]
            # fp8 DoubleRow, one matmul per adjacent m-tile pair.
            # z_all[c] row k*P+p holds (t, o) = z[k*ROWS + c*RPC + t*P+p]
            # -> m-tile of (c, k, t) is 8k + 4c + t.
            outT_sb = outsb_p.tile([P, OB, ROWS], f32)
            with tc.tile_pool(name="ps4", bufs=1, space="PSUM") as ps4:
                psum_o = [[ps4.tile([P, HF], f32, name=f"po{ob}_{ih}",
                                    tag=f"po{ob}_{ih}")
                           for ih in range(IH)] for ob in range(OB)]
                first = True
                for c in range(GC):
                    for k in range(NCORES):
                        zc_sb = zchunk_p.tile([P, QT, OUTD], f8,
                                              tag="zchunk")
                        nc.gpsimd.dma_start(
                            zc_sb[:], z_all[c][k * P:(k + 1) * P, :])
                        last_grp = (c == GC - 1 and k == NCORES - 1)
                        a_prs = {}
                        for pr in range(0, QT, 2):
                            mt = IB * k + QT * c + pr
                            a_pair = apair_p.tile([P, 2, ROWS], f8,
                                                  tag="apair")
                            nc.scalar.dma_start(
                                a_pair[:],
                                adjP_e[(mt // 2) * P:(mt // 2 + 1) * P, :])
                            a_prs[pr] = a_pair
                        # last group runs (ob, ih)-outer so each output
                        # quarter stops as early as possible and its BN
                        # eviction + store overlap the remaining matmuls
                        if last_grp:
                            for ob in range(OB):
                                for ih in range(IH):
                                    for pr in range(0, QT, 2):
                                        nc.tensor.matmul(
                                            psum_o[ob][ih][:],
                                            zc_sb[:, pr:pr + 2,
                                                  ob * P:(ob + 1) * P],
                                            a_prs[pr][:, :,
                                                      ih * HF:
                                                      (ih + 1) * HF],
                                            start=False,
                                            stop=(pr == QT - 2),
                                            perf_mode=DR,
                                        )
                        else:
                            for pr in range(0, QT, 2):
                                for ob in range(OB):
                                    for ih in range(IH):
                                        nc.tensor.matmul(
                                            psum_o[ob][ih][:],
                                            zc_sb[:, pr:pr + 2,
                                                  ob * P:(ob + 1) * P],
                                            a_prs[pr][:, :,
                                                      ih * HF:
                                                      (ih + 1) * HF],
                                            start=first, stop=False,
                                            perf_mode=DR,
                                        )
                                first = False
                # fused BN affine on PSUM evict: out = psum*scale + bias;
                # store per quarter so the tail pipelines
                for ob in range(OB):
                    for ih in range(IH):
                        nc.vector.tensor_scalar(
                            outT_sb[:, ob, ih * HF:(ih + 1) * HF],
                            psum_o[ob][ih][:],
                            bnsc_sb[:, ob:ob + 1],
                            bnbi_sb[:, ob:ob + 1],
                            mybir.AluOpType.mult,
                            mybir.AluOpType.add)
                        nc.sync.dma_start(
                            out_e[ob * P:(ob + 1) * P,
                                  ih * HF:(ih + 1) * HF],
                            outT_sb[:, ob, ih * HF:(ih + 1) * HF])

    nc.compile()
    return nc


def _get_nc():
    if "nc" not in _cache:
        _cache["nc"] = _build()
    return _cache["nc"]


def kernel(x, IFadj, adj, W1, b1, W2, b2, bn_gamma, bn_beta, bn_mean, bn_var):
    from concourse.bass_utils import run_bass_kernel_spmd

    x = np.asarray(x, dtype=np.float32)
    IFadj = np.asarray(IFadj, dtype=np.float32)
    adj = np.asarray(adj, dtype=np.float32)
    W1 = np.asarray(W1, dtype=np.float32)
    b1 = np.asarray(b1, dtype=np.float32)
    W2 = np.asarray(W2, dtype=np.float32)
    b2 = np.asarray(b2, dtype=np.float32)
    bn_gamma = np.asarray(bn_gamma, dtype=np.float32)
    bn_beta = np.asarray(bn_beta, dtype=np.float32)
    bn_mean = np.asarray(bn_mean, dtype=np.float32)
    bn_var = np.asarray(bn_var, dtype=np.float32)

    # host-side prep: shard rows, transpose for PE lhsT layout, cast.
    # W2 is pre-scaled by 1/4 so z stays well inside fp8e4 range; the
    # BN scale is multiplied by 4 to undo it after the layer-2 spmm.
    w1b = np.ascontiguousarray(
        W1.astype(_BF16).reshape(CB, P, NHID).transpose(1, 0, 2)
        .reshape(P, CB * NHID))
    w2b = np.ascontiguousarray(
        (W2 * 0.25).astype(_BF16).reshape(JB, P, OUTD)
        .transpose(1, 0, 2).reshape(P, JB * OUTD))
    # layer-1 bias including the exact 1/2*colsum(S) centering term
    colsum = x.sum(axis=0, dtype=np.float64).astype(np.float32) @ W1
    b1c = b1 + 0.5 * colsum
    b1p = np.ascontiguousarray(b1c.reshape(JB, P).T)  # [P, JB]
    inv = bn_gamma / np.sqrt(bn_var + BN_EPS)
    bias_tot = b2 * inv + bn_beta - bn_mean * inv
    bnsc = np.ascontiguousarray((4.0 * inv).reshape(OB, P).T)   # [P, OB]
    bnbi = np.ascontiguousarray(bias_tot.reshape(OB, P).T)      # [P, OB]

    # replicated x rows for global node blocks 5..7, partition-major
    xTr = np.ascontiguousarray(
        x[RED0 * ROWS:].T.astype(_BF16).reshape(CB, P, NRED * ROWS)
        .transpose(1, 0, 2).reshape(P, CB * NRED * ROWS))

    in_maps = []
    for k in range(NCORES):
        r0, r1 = k * ROWS, (k + 1) * ROWS
        # centered IFadj^T in fp8: [m, col] -> [ih, pair, p, t, c]
        A8 = (IFadj[r0:r1].T - np.float32(0.5)).astype(_F8)  # [N, ROWS]
        ifadjH = np.ascontiguousarray(
            A8.reshape(N // 256, 2, P, IH, HF).transpose(3, 0, 2, 1, 4)
            .reshape(IH * N // 2, 2 * HF))
        adjT8 = np.ascontiguousarray(adj[r0:r1].T).astype(_F8)  # [N, ROWS]
        # pair-interleave: row pair*P+p = m-tiles (2p, 2p+1) side by side
        adjP = np.ascontiguousarray(
            adjT8.reshape(N // 256, 2, P, ROWS).transpose(0, 2, 1, 3)
            .reshape(N // 2, 2 * ROWS))
        xTp = np.ascontiguousarray(
            x[r0:r1].T.astype(_BF16).reshape(CB, P, ROWS)
            .transpose(1, 0, 2).reshape(P, CB * ROWS))
        in_maps.append({
            "xT": xTp,
            "xTr": xTr,
            "ifadjH": ifadjH,
            "adjP": adjP,
            "w1": w1b,
            "w2": w2b,
            "b1p": b1p,
            "bnsc": bnsc,
            "bnbi": bnbi,
        })

    global _last_in_maps
    _last_in_maps = in_maps

    nc = _get_nc()
    try:
        res = run_bass_kernel_spmd(nc, in_maps, list(range(NCORES)))
    except Exception:
        # transient device wedge (NRT_EXEC_UNIT_UNRECOVERABLE etc.) --
        # a straight retry has been observed to recover
        import time
        time.sleep(2.0)
        res = run_bass_kernel_spmd(nc, in_maps, list(range(NCORES)))
    # per-core output is outT [OUTD, ROWS]; transpose back and stack rows
    return np.concatenate(
        [np.ascontiguousarray(res.results[k]["out"].T)
         for k in range(NCORES)], axis=0)
